# revision 68
# baseline (speedup 1.0000x reference)
"""Trainium2 Bass kernel for nn_NexusV2 (CentroidAddressableManifold.read).

Strategy: shard by *bucket*. Tokens are routed host-side to the core owning
their bucket; each occupied bucket's 32 slot rows stream from HBM exactly
once, in fp16 (vs. the reference's per-token f32 gather).

Device layout (per core, all shapes static at trace time):
  - 8 groups, each holding <=8 buckets; tokens of a bucket occupy a
    contiguous run of rows (no fixed padding); rows per group NT[g] is the
    max over cores for that group slot (SPMD shares one NEFF).
  - per group: PE computes u = q@[K|a] (fp16, token-major), one blend
    matmul adds the (window-masked) anchor-dot table, softmax + hard-match
    on DVE/ACT (Scalar runs only Rsqrt/Exp tables), val = probs @ V on PE.
  - |q| is folded into the softmax scale: no on-device q normalization or
    transposes (host supplies raw q both row-major and D-major).
  - hard-match windowing is folded into the tid encoding host-side
    (tid + window*2^17, exact in f32), so no mask tiles are built.

Host does only routing/permutation + packing of the read-only tables; all
FLOPs of the reference (norms, dots, softmax, matches, matmuls) run on
device.
"""

import sys
import types

import numpy as np

N_BUCKETS = 512
SPB = 32          # slots per bucket
TAU = 0.1
BPG = 8           # buckets per group
NGRP = 8          # groups per core
N_CORES = 8
D = 1024
KCH = 8           # D / 128 contraction chunks
NS = SPB * BPG    # 256 slot columns per group
NSP = NS + BPG
APAD = 32         # token base partition inside u_ps (PSUM quadrant align)
NEG = -30000.0    # additive mask value
WENC = 131072.0   # 2^17 window encoding for tid match

_COMPILED = {}    # plan -> nc
_HOOK_DONE = False


# ----------------------------------------------------------------- utilities

def _install_ntff_hook():
    """Synthesize antenv.axon_hooks so trace=True can NTFF-profile (optional)."""
    global _HOOK_DONE
    if _HOOK_DONE or 'antenv.axon_hooks' in sys.modules:
        _HOOK_DONE = True
        return
    try:
        import antenv
        m = types.ModuleType('antenv.axon_hooks')
        _hook = [None]
        m.set_axon_ntff_profile_hook = lambda h: _hook.__setitem__(0, h)
        m.get_axon_ntff_profile_hook = lambda: _hook[0]
        sys.modules['antenv.axon_hooks'] = m
        antenv.axon_hooks = m
        if '/root/.axon_site' not in sys.path:
            sys.path.insert(0, '/root/.axon_site')
        from trn_agent_boot.trn_boot import _ntff_profile_via_ctypes
        m.set_axon_ntff_profile_hook(
            _ntff_profile_via_ctypes('/opt/axon/libaxon_pjrt.so'))
    except Exception:
        pass
    _HOOK_DONE = True


def _routing(tids_flat):
    """Pack occupied buckets into NGRP*N_CORES bins (<=BPG buckets each),
    LPT-balanced by token count. Returns (bins, NT, tok_of_bucket) where
    bins[g][c] = bucket ids for core c / group-slot g, NT[g] = padded row
    count of slot g."""
    buckets = tids_flat.astype(np.int64) % N_BUCKETS
    order = np.argsort(buckets, kind='stable')
    counts = np.bincount(buckets, minlength=N_BUCKETS)
    cum = np.concatenate([[0], np.cumsum(counts)])
    tok_of_bucket = {b: order[cum[b]:cum[b + 1]]
                     for b in range(N_BUCKETS) if counts[b] > 0}

    n_bins = NGRP * N_CORES
    occ = sorted(tok_of_bucket, key=lambda b: -counts[b])
    bin_rows = np.zeros(n_bins, np.int64)
    bin_cnt = np.zeros(n_bins, np.int64)
    bin_members = [[] for _ in range(n_bins)]
    for b in occ:
        cand = np.where(bin_cnt < BPG)[0]
        i = cand[np.argmin(bin_rows[cand])]
        bin_members[i].append(b)
        bin_rows[i] += counts[b]
        bin_cnt[i] += 1
    assert bin_rows.max() <= 128 - APAD, "group row overflow"
    # sort bins by rows desc; slot g = bins[8g:8g+8]; snake over cores
    srt = np.argsort(-bin_rows, kind='stable')
    bins, NT = [], []
    for g in range(NGRP):
        sel = [srt[g * N_CORES + c] for c in range(N_CORES)]
        sl = [bin_members[i] for i in sel]
        if g % 2:
            sl = sl[::-1]
        bins.append(sl)
        mx = max(2, max(bin_rows[i] for i in sel))
        NT.append(int(min(128 - APAD, (mx + 1) // 2 * 2)))
    return bins, NT, tok_of_bucket


def _geom(NT):
    """Per-group kv column geometry: KCH chunks of [K^T_k | a^T_k]
    (NSP cols each), oh8h block (NT[g]+APAD cols), then 2 V halves.
    qTa chunks are [a^T_k | pad | qT_k] (NT[g]+APAD cols each); token
    rows sit at partition APAD.. everywhere before the probs transpose."""
    wgs = [KCH * NSP + NT[g] + APAD + 2 * D for g in range(NGRP)]
    kcols = np.concatenate([[0], np.cumsum(wgs)]).astype(int)
    rows = np.concatenate([[0], np.cumsum(NT)]).astype(int)
    rowsP = np.concatenate([[0], np.cumsum([n + APAD for n in NT])]).astype(int)
    qtc = np.concatenate(
        [[0], np.cumsum([KCH * (n + APAD) for n in NT])]).astype(int)
    return wgs, kcols, rows, rowsP, qtc


def _consts():
    iota8 = np.broadcast_to(np.arange(BPG, dtype=np.float32),
                            (128, BPG)).copy()
    win = (np.arange(NS)[None, :] // SPB) == np.arange(BPG)[:, None]
    winmask8 = np.where(win, 0.0, NEG).astype(np.float32)      # [BPG, NS]
    identw = np.eye(128, dtype=np.float16)
    return iota8, winmask8, identw


def _pack_core(core_bins, NT, tok_of_bucket, q_flat, tids_flat,
               KT, V, slot_tids, CBT):
    """Build this core's input arrays. core_bins[g] = list of bucket ids."""
    wgs, kcols, rows, rowsP, qtc = _geom(NT)
    i16 = int(rows[-1])
    qrow = np.zeros((int(rowsP[-1]), D), np.float16)
    qT = np.zeros((128, int(qtc[-1])), np.float16)
    side = np.full((128, 2 * NGRP), -1.0, np.float32)
    tidb = np.full((1, NGRP * NS), -2.0, np.float32)
    kv = np.zeros((128, int(kcols[-1])), np.float16)
    tok_idx = np.full(i16, -1, np.int64)

    for g in range(NGRP):
        nt, col = NT[g], int(kcols[g])
        ntp = nt + APAD
        slot_ids = np.zeros(NS, np.int64)
        real_slots = np.zeros(NS, bool)
        anchors = np.zeros((D, BPG), np.float32)
        oh8h = np.zeros((BPG, ntp), np.float32)
        qTv = qT[:, int(qtc[g]):int(qtc[g]) + KCH * ntp].reshape(128, KCH, ntp)
        r = 0
        for j, b in enumerate(core_bins[g]):
            toks = tok_of_bucket[b]
            c = len(toks)
            slot_ids[j * SPB:(j + 1) * SPB] = np.arange(b * SPB, (b + 1) * SPB)
            real_slots[j * SPB:(j + 1) * SPB] = True
            anchors[:, j] = CBT[:, b]
            tidb[0, g * NS + j * SPB:g * NS + (j + 1) * SPB] = \
                slot_tids[b * SPB:(b + 1) * SPB] + j * WENC
            qg = q_flat[toks]
            qrow[int(rowsP[g]) + APAD + r:int(rowsP[g]) + APAD + r + c] = qg
            qTv[:, :, APAD + r:APAD + r + c] = \
                qg.reshape(c, KCH, 128).transpose(2, 1, 0)
            side[APAD + r:APAD + r + c, 2 * g] = tids_flat[toks] + j * WENC
            side[APAD + r:APAD + r + c, 2 * g + 1] = j
            oh8h[j, APAD + r:APAD + r + c] = 0.5
            tok_idx[int(rows[g]) + r:int(rows[g]) + r + c] = toks
            r += c
        # anchors as leading stationary columns (a0t rows 0..7 in u_ps)
        qTv[:, :, 0:BPG] = anchors.reshape(KCH, 128, BPG).transpose(1, 0, 2)
        # ka chunks [KCH, 128, NSP]: per chunk K^T slots then anchors
        ktg = KT[:, slot_ids].reshape(KCH, 128, NS) * real_slots[None, None, :]
        atp = anchors.reshape(KCH, 128, BPG)
        ka = np.concatenate([ktg, atp], axis=2)
        c0 = col
        kv[:, c0:c0 + KCH * NSP] = \
            ka.transpose(1, 0, 2).reshape(128, -1).astype(np.float16)
        c0 += KCH * NSP
        kv[0:BPG, c0:c0 + ntp] = oh8h.astype(np.float16)
        c0 += ntp
        vb = V[slot_ids] * real_slots[:, None]          # [NS, D]
        kv[:, c0:c0 + D] = vb[0:128].astype(np.float16)
        kv[:, c0 + D:c0 + 2 * D] = vb[128:256].astype(np.float16)
    return dict(qrow=qrow, qT=qT, side=side, tidb=tidb, kv=kv), tok_idx


# ------------------------------------------------------------- device kernel

def _build_nc(NT):
    from concourse import bacc, mybir, tile

    F32 = mybir.dt.float32
    F16 = mybir.dt.float16
    AL = mybir.AluOpType
    AF = mybir.ActivationFunctionType
    X = mybir.AxisListType.X

    wgs, kcols, rows, rowsP, qtc = _geom(NT)
    i16 = int(rows[-1])

    nc = bacc.Bacc(trn_type="TRN2", target_bir_lowering=False, debug=False)
    d_kv = nc.dram_tensor("kv", [128, int(kcols[-1])], F16,
                          kind="ExternalInput").ap()
    d_qT = nc.dram_tensor("qT", [128, int(qtc[-1])], F16,
                          kind="ExternalInput").ap()
    d_qrow = nc.dram_tensor("qrow", [int(rowsP[-1]), D], F16,
                            kind="ExternalInput").ap()
    d_side = nc.dram_tensor("side", [128, 2 * NGRP], F32,
                            kind="ExternalInput").ap()
    d_tidb = nc.dram_tensor("tidb", [1, NGRP * NS], F32,
                            kind="ExternalInput").ap()
    d_iota8 = nc.dram_tensor("iota8", [128, BPG], F32,
                             kind="ExternalInput").ap()
    d_winmask8 = nc.dram_tensor("winmask8", [BPG, NS], F32,
                                kind="ExternalInput").ap()
    d_identw = nc.dram_tensor("identw", [128, 128], F16,
                              kind="ExternalInput").ap()
    d_out = nc.dram_tensor("outp", [i16, D], F16, kind="ExternalOutput").ap()
    w2max = max(wgs[2 * p] + wgs[2 * p + 1] for p in range(NGRP // 2))
    qt2max = max(int(qtc[2 * p + 2] - qtc[2 * p]) for p in range(NGRP // 2))

    with tile.TileContext(nc) as tc:
        with tc.tile_pool(name="const", bufs=1) as pc, \
             tc.tile_pool(name="kvp", bufs=4) as pkv, \
             tc.tile_pool(name="io", bufs=4) as pio, \
             tc.tile_pool(name="wk", bufs=2) as pw, \
             tc.tile_pool(name="outp", bufs=3) as po, \
             tc.tile_pool(name="psA", bufs=2, space="PSUM") as ppa, \
             tc.tile_pool(name="psB", bufs=1, space="PSUM") as ppb:

            iota8 = pc.tile([128, BPG], F32)
            winmask8 = pc.tile([BPG, NS], F32)
            identw = pc.tile([128, 128], F16)
            side_t = pc.tile([128, 2 * NGRP], F32)
            tidb_a = pc.tile([1, NGRP * NS], F32)
            nc.sync.dma_start(iota8[:], d_iota8)
            nc.sync.dma_start(winmask8[:], d_winmask8)
            nc.sync.dma_start(identw[:], d_identw)
            nc.sync.dma_start(side_t[:], d_side)
            nc.sync.dma_start(tidb_a[:], d_tidb)
            eps6 = pc.tile([128, 1], F32)
            nc.gpsimd.memset(eps6[:], 1e-6)
            tau28 = pc.tile([128, 1], F32)
            nc.gpsimd.memset(tau28[:], TAU * TAU / 8.0)
            ones256 = pc.tile([128, NS], F32)
            nc.gpsimd.memset(ones256[:], 1.0)

            kv2_t = qt2_t = None
            for g in range(NGRP):
                nt = NT[g]
                ntp = nt + APAD
                wg = wgs[g]
                r0 = int(rows[g])
                r0p = int(rowsP[g])
                voff = KCH * NSP + ntp  # V offset within this group's kv
                B0, B1 = APAD, APAD + nt  # token rows (anchors/pad at 0:32)

                if g % 2 == 0:
                    wpair = wgs[g] + wgs[g + 1]
                    qpair = int(qtc[g + 2] - qtc[g])
                    kv2_t = pkv.tile([128, w2max], F16, tag="kv")
                    nc.sync.dma_start(kv2_t[:, 0:wpair],
                                      d_kv[:, int(kcols[g]):
                                           int(kcols[g]) + wpair])
                    qt2_t = pio.tile([128, qt2max], F16, tag="qt")
                    nc.scalar.dma_start(qt2_t[:, 0:qpair],
                                        d_qT[:, int(qtc[g]):
                                             int(qtc[g]) + qpair])
                    kvo = qto = 0
                else:
                    kvo = wgs[g - 1]
                    qto = int(qtc[g] - qtc[g - 1])
                kv_t = kv2_t[:, kvo:kvo + wg]
                ka = kv_t[:, 0:KCH * NSP].rearrange("p (k s) -> p k s", k=KCH)
                qta = qt2_t[:, qto:qto + KCH * ntp].rearrange(
                    "p (k t) -> p k t", k=KCH)
                qr_t = pio.tile([128, D], F16, tag="qr")
                nc.scalar.dma_start(qr_t[0:B1, :], d_qrow[r0p:r0p + B1, :])
                tidb_t = pw.tile([128, NS], F32, tag="tidb")
                nc.gpsimd.partition_broadcast(
                    tidb_t[0:B1, :], tidb_a[0:1, g * NS:(g + 1) * NS])

                # --- mask8: row's own instance column (for q.a extraction)
                mask8 = pw.tile([128, BPG], F32, tag="mask8")
                nc.vector.tensor_scalar(out=mask8[0:B1, :],
                                        in0=iota8[0:B1, :],
                                        scalar1=side_t[0:B1,
                                                       2 * g + 1:2 * g + 2],
                                        scalar2=None, op0=AL.is_equal)

                # --- ssq = |q|^2 ; s1 = 0.5/|q| = rsqrt(4 ssq)  (GpSimd)
                scr = pw.tile([128, D], F16, tag="scr")
                ssq = pw.tile([128, 1], F32, tag="ssq")
                nc.vector.scalar_tensor_tensor(
                    out=scr[0:B1, :], in0=qr_t[0:B1, :], scalar=1.0,
                    in1=qr_t[0:B1, :], op0=AL.mult, op1=AL.mult,
                    accum_out=ssq[0:B1, :])
                s1n = pw.tile([128, 1], F32, tag="s1n")
                nc.scalar.activation(s1n[0:B1, :], ssq[0:B1, :], AF.Ln,
                                     bias=eps6[0:B1, :], scale=4.0)
                s1 = pw.tile([128, 1], F32, tag="s1")
                nc.scalar.activation(s1[0:B1, :], s1n[0:B1, :], AF.Exp,
                                     scale=-0.5)

                # --- u rows: [a0t(8) | tokens(nt)] x [K | a]
                u_ps = ppa.tile([128, NSP], F32, tag="u")
                for k in range(KCH):
                    nc.tensor.matmul(u_ps[0:ntp, :], qta[:, k, 0:ntp],
                                     ka[:, k, :], start=(k == 0),
                                     stop=(k == KCH - 1))
                a0t = pw.tile([BPG, NS], F16, tag="a0tsb")
                nc.vector.scalar_tensor_tensor(
                    out=a0t[:], in0=u_ps[0:BPG, 0:NS], scalar=1.0,
                    in1=winmask8[:], op0=AL.mult, op1=AL.add)
                c_ps = ppa.tile([128, NS], F32, tag="c")
                nc.tensor.matmul(c_ps[0:B1, :],
                                 kv_t[0:BPG, KCH * NSP:KCH * NSP + ntp],
                                 a0t[:], start=True, stop=True)

                # --- rw10 = (1/W)/tau = recip(sqrt(4*(tau^2/4)*W^2))
                qasc = pw.tile([128, BPG], F32, tag="qasc")
                qa1 = pw.tile([128, 1], F32, tag="qa1")
                nc.vector.scalar_tensor_tensor(
                    out=qasc[0:B1, :], in0=u_ps[0:B1, NS:NSP],
                    scalar=TAU * TAU / 4.0,
                    in1=mask8[0:B1, :], op0=AL.mult, op1=AL.mult,
                    accum_out=qa1[0:B1, :])
                w2 = pw.tile([128, 1], F32, tag="w2")
                nc.vector.scalar_tensor_tensor(
                    out=w2[0:B1, :], in0=qa1[0:B1, :], scalar=s1[0:B1, :],
                    in1=tau28[0:B1, :], op0=AL.mult, op1=AL.add)
                rwn = pw.tile([128, 1], F32, tag="rwn")
                nc.scalar.activation(rwn[0:B1, :], w2[0:B1, :], AF.Ln,
                                     bias=eps6[0:B1, :], scale=4.0)
                rw10 = pw.tile([128, 1], F32, tag="rw10")
                nc.scalar.activation(rw10[0:B1, :], rwn[0:B1, :], AF.Exp,
                                     scale=-0.5)

                # --- sc = u*s1 + c   (masked cols ~ -15000)
                sc1 = pw.tile([128, NS], F32, tag="sc1")
                nc.vector.tensor_scalar(out=sc1[0:B1, :],
                                        in0=u_ps[0:B1, 0:NS],
                                        scalar1=s1[0:B1, :], scalar2=None,
                                        op0=AL.mult)
                sc = pw.tile([128, NS], F32, tag="sc")
                nc.vector.tensor_tensor(out=sc[0:B1, :], in0=sc1[0:B1, :],
                                        in1=c_ps[0:B1, :], op=AL.add)
                negmax = pw.tile([128, 1], F32, tag="negmax")
                nc.vector.reduce_max(negmax[0:B1, :], sc[0:B1, :], axis=X,
                                     negate=True)
                ebias = pw.tile([128, 1], F32, tag="ebias")
                nc.vector.tensor_tensor(out=ebias[0:B1, :],
                                        in0=negmax[0:B1, :],
                                        in1=rw10[0:B1, :], op=AL.mult)
                ex = pw.tile([128, NS], F32, tag="ex")
                esum = pw.tile([128, 1], F32, tag="esum")
                nc.scalar.activation(ex[0:B1, :], sc[0:B1, :], AF.Exp,
                                     bias=ebias[0:B1, :],
                                     scale=rw10[0:B1, :],
                                     accum_out=esum[0:B1, :])
                rsum = pw.tile([128, 1], F32, tag="rsum")
                nc.vector.reciprocal(rsum[0:B1, :], esum[0:B1, :])

                # --- hard match path (window folded into tid encoding)
                match = pw.tile([128, NS], F32, tag="match")
                msum = pw.tile([128, 1], F32, tag="msum")
                nc.vector.scalar_tensor_tensor(
                    out=match[0:B1, :], in0=tidb_t[0:B1, :],
                    scalar=side_t[0:B1, 2 * g:2 * g + 1],
                    in1=ones256[0:B1, :],
                    op0=AL.is_equal, op1=AL.mult,
                    accum_out=msum[0:B1, :])
                mden = pw.tile([128, 1], F32, tag="mden")
                nc.vector.tensor_scalar(out=mden[0:B1, :], in0=msum[0:B1, :],
                                        scalar1=1e-9, scalar2=None,
                                        op0=AL.add)
                mrec = pw.tile([128, 1], F32, tag="mrec")
                nc.vector.reciprocal(mrec[0:B1, :], mden[0:B1, :])
                nohas = pw.tile([128, 1], F32, tag="nohas")
                nc.vector.tensor_scalar(out=nohas[0:B1, :],
                                        in0=msum[0:B1, :],
                                        scalar1=0.0, scalar2=None,
                                        op0=AL.is_le)
                rs_nh = pw.tile([128, 1], F32, tag="rs_nh")
                nc.vector.tensor_tensor(out=rs_nh[0:B1, :],
                                        in0=rsum[0:B1, :],
                                        in1=nohas[0:B1, :], op=AL.mult)
                hard = pw.tile([128, NS], F32, tag="hard")
                nc.vector.tensor_scalar(out=hard[0:B1, :],
                                        in0=match[0:B1, :],
                                        scalar1=mrec[0:B1, :], scalar2=None,
                                        op0=AL.mult)
                probs = pw.tile([128, NS], F16, tag="probs")
                nc.vector.scalar_tensor_tensor(
                    out=probs[0:B1, :], in0=ex[0:B1, :],
                    scalar=rs_nh[0:B1, :], in1=hard[0:B1, :],
                    op0=AL.mult, op1=AL.add)

                # --- probs^T (junk rows 0:32 become junk cols), val=probs@V
                pt_ps = ppb.tile([128, 2, 128], F16, tag="pt")
                for h in range(2):
                    nc.tensor.transpose(pt_ps[:, h, 0:B1],
                                        probs[0:B1, h * 128:(h + 1) * 128],
                                        identw[0:B1, 0:B1])
                pt = pw.tile([128, 2, 128], F16, tag="ptsb")
                for h in range(2):
                    nc.vector.tensor_copy(pt[:, h, B0:B1],
                                          pt_ps[:, h, B0:B1])
                pv = ppb.tile([128, D], F32, tag="pv")
                for j in range(2):
                    for h in range(2):
                        nc.tensor.matmul(
                            pv[0:nt, j * 512:(j + 1) * 512],
                            pt[:, h, B0:B1],
                            kv_t[:, voff + h * D + j * 512:
                                 voff + h * D + (j + 1) * 512],
                            start=(h == 0), stop=(h == 1))
                out_sb = po.tile([128, D], F16, tag="out_sb")
                nc.vector.tensor_copy(out_sb[0:nt, :], pv[0:nt, :])
                nc.sync.dma_start(d_out[r0:r0 + nt, :], out_sb[0:nt, :])
    nc.compile()
    return nc


# ------------------------------------------------------------------ emulator

def _emulate_core(ins, NT):
    """Numpy emulation of the device kernel (fp16 data, f32 math)."""
    qrow = ins["qrow"].astype(np.float32)
    qT = ins["qT"].astype(np.float32)
    side, tidb = ins["side"], ins["tidb"]
    kv = ins["kv"].astype(np.float32)
    wgs, kcols, rows, rowsP, qtc = _geom(NT)
    i16 = int(rows[-1])
    out = np.zeros((i16, D), np.float32)
    iota8, winmask8, _ = _consts()
    for g in range(NGRP):
        nt, col = NT[g], int(kcols[g])
        ntp = nt + APAD
        voff = col + KCH * NSP + ntp
        ka = kv[:, col:col + KCH * NSP].reshape(128, KCH, NSP)
        KT = ka[:, :, 0:NS].transpose(1, 0, 2).reshape(D, NS)
        AT = ka[:, :, NS:NSP].transpose(1, 0, 2).reshape(D, BPG)
        oh8h = kv[0:BPG, col + KCH * NSP:col + KCH * NSP + ntp][:, APAD:]
        vb = np.concatenate([kv[:, voff:voff + D].T,
                             kv[:, voff + D:voff + 2 * D].T], axis=1).T
        q = qrow[int(rowsP[g]) + APAD:int(rowsP[g]) + ntp]
        qTg = qT[:, int(qtc[g]):int(qtc[g]) + KCH * ntp].reshape(128, KCH,
                                                                 ntp)
        qTg = qTg.transpose(1, 0, 2).reshape(D, ntp)[:, APAD:]
        sd = side[APAD:APAD + nt, 2 * g:2 * g + 2]

        mask8 = (iota8[0:nt, :] == sd[:, 1:2]).astype(np.float32)
        ssq = (q * q).sum(-1, keepdims=True)
        s1 = 1.0 / np.sqrt(4.0 * ssq + 1e-6)
        a0t = (AT.T @ KT + winmask8).astype(np.float16).astype(np.float32)
        u = qTg.T @ np.concatenate([KT, AT], axis=1)    # [nt, NSP]
        c = oh8h.T @ a0t
        qa1 = (u[:, NS:NSP] * (TAU * TAU / 4.0) * mask8).sum(-1,
                                                             keepdims=True)
        w2 = qa1 * s1 + TAU * TAU / 8.0
        rw10 = 1.0 / np.sqrt(4.0 * w2 + 1e-6)
        sc = u[:, 0:NS] * s1 + c
        m = sc.max(-1, keepdims=True)
        ex = np.exp((sc - m) * rw10)
        esum = ex.sum(-1, keepdims=True)
        match = (tidb[0, g * NS:(g + 1) * NS][None, :] ==
                 sd[:, 0:1]).astype(np.float32)
        msum = match.sum(-1, keepdims=True)
        nohas = (msum <= 0).astype(np.float32)
        hard = match / (msum + 1e-9)
        probs = (ex * (nohas / esum) + hard).astype(np.float16)
        out[rows[g]:rows[g] + nt] = \
            (probs.astype(np.float32) @ vb).astype(np.float16)
    return out


# -------------------------------------------------------------------- kernel

def kernel(query_emb, tids, slot_keys, slot_values, slot_tids,
           centroid_codebook, _emulate=False, _trace=False):
    B, T, _ = query_emb.shape
    BT = B * T
    q_flat = np.ascontiguousarray(query_emb.reshape(BT, D), np.float32)
    tids_flat = np.asarray(tids).reshape(BT)
    st = np.asarray(slot_tids).astype(np.float32)
    KT = np.ascontiguousarray(np.asarray(slot_keys, np.float32).T)     # [D, S]
    V = np.asarray(slot_values, np.float32)
    CBT = np.ascontiguousarray(np.asarray(centroid_codebook, np.float32).T)

    bins, NT, tok_of_bucket = _routing(tids_flat)
    iota8, winmask8, identw = _consts()

    in_maps, tok_idxs = [], []
    for c in range(N_CORES):
        core_bins = [bins[g][c] for g in range(NGRP)]
        ins, tok_idx = _pack_core(core_bins, NT, tok_of_bucket, q_flat,
                                  tids_flat, KT, V, st, CBT)
        ins.update(iota8=iota8, winmask8=winmask8, identw=identw)
        in_maps.append(ins)
        tok_idxs.append(tok_idx)

    out_flat = np.zeros((BT, D), np.float32)
    if _emulate:
        for c in range(N_CORES):
            o = _emulate_core(in_maps[c], NT)
            valid = tok_idxs[c] >= 0
            out_flat[tok_idxs[c][valid]] = o[valid]
        return out_flat.reshape(B, T, D).astype(np.float32)

    _install_ntff_hook()
    from concourse import bass_utils
    key = tuple(NT)
    if key not in _COMPILED:
        _COMPILED[key] = _build_nc(NT)
    nc = _COMPILED[key]
    res = bass_utils.run_bass_kernel_spmd(
        nc, in_maps, core_ids=list(range(N_CORES)), trace=_trace)
    for c in range(N_CORES):
        o = np.asarray(res.results[c]["outp"], np.float32)
        valid = tok_idxs[c] >= 0
        out_flat[tok_idxs[c][valid]] = o[valid]
    out = out_flat.reshape(B, T, D).astype(np.float32)
    if _trace:
        kernel._last_exec_time_ns = res.exec_time_ns
        kernel._last_results = res
    return out


# revision 69
# speedup vs baseline: 1.0349x; 1.0349x over previous
"""Trainium2 Bass kernel for nn_NexusV2 (CentroidAddressableManifold.read).

Strategy: shard by *bucket*. Tokens are routed host-side to the core owning
their bucket; each occupied bucket's 32 slot rows stream from HBM exactly
once, in fp16 (vs. the reference's per-token f32 gather).

Device layout (per core, all shapes static at trace time):
  - 8 groups, each holding <=8 buckets; tokens of a bucket occupy a
    contiguous run of rows (no fixed padding); rows per group NT[g] is the
    max over cores for that group slot (SPMD shares one NEFF).
  - per group: PE computes u = q@[K|a] (fp16, token-major), one blend
    matmul adds the (window-masked) anchor-dot table, softmax + hard-match
    on DVE/ACT (Scalar runs only Rsqrt/Exp tables), val = probs @ V on PE.
  - |q| is folded into the softmax scale: no on-device q normalization or
    transposes (host supplies raw q both row-major and D-major).
  - hard-match windowing is folded into the tid encoding host-side
    (tid + window*2^17, exact in f32), so no mask tiles are built.

Host does only routing/permutation + packing of the read-only tables; all
FLOPs of the reference (norms, dots, softmax, matches, matmuls) run on
device.
"""

import sys
import types

import numpy as np

N_BUCKETS = 512
SPB = 32          # slots per bucket
TAU = 0.1
BPG = 8           # buckets per group
NGRP = 8          # groups per core
N_CORES = 8
D = 1024
KCH = 8           # D / 128 contraction chunks
NS = SPB * BPG    # 256 slot columns per group
NSP = NS + BPG
APAD = 32         # token base partition inside u_ps (PSUM quadrant align)
NEG = -30000.0    # additive mask value
WENC = 131072.0   # 2^17 window encoding for tid match

_COMPILED = {}    # plan -> nc
_HOOK_DONE = False


# ----------------------------------------------------------------- utilities

def _install_ntff_hook():
    """Synthesize antenv.axon_hooks so trace=True can NTFF-profile (optional)."""
    global _HOOK_DONE
    if _HOOK_DONE or 'antenv.axon_hooks' in sys.modules:
        _HOOK_DONE = True
        return
    try:
        import antenv
        m = types.ModuleType('antenv.axon_hooks')
        _hook = [None]
        m.set_axon_ntff_profile_hook = lambda h: _hook.__setitem__(0, h)
        m.get_axon_ntff_profile_hook = lambda: _hook[0]
        sys.modules['antenv.axon_hooks'] = m
        antenv.axon_hooks = m
        if '/root/.axon_site' not in sys.path:
            sys.path.insert(0, '/root/.axon_site')
        from trn_agent_boot.trn_boot import _ntff_profile_via_ctypes
        m.set_axon_ntff_profile_hook(
            _ntff_profile_via_ctypes('/opt/axon/libaxon_pjrt.so'))
    except Exception:
        pass
    _HOOK_DONE = True


def _routing(tids_flat):
    """Pack occupied buckets into NGRP*N_CORES bins (<=BPG buckets each),
    LPT-balanced by token count. Returns (bins, NT, tok_of_bucket) where
    bins[g][c] = bucket ids for core c / group-slot g, NT[g] = padded row
    count of slot g."""
    buckets = tids_flat.astype(np.int64) % N_BUCKETS
    order = np.argsort(buckets, kind='stable')
    counts = np.bincount(buckets, minlength=N_BUCKETS)
    cum = np.concatenate([[0], np.cumsum(counts)])
    tok_of_bucket = {b: order[cum[b]:cum[b + 1]]
                     for b in range(N_BUCKETS) if counts[b] > 0}

    n_bins = NGRP * N_CORES
    occ = sorted(tok_of_bucket, key=lambda b: -counts[b])
    bin_rows = np.zeros(n_bins, np.int64)
    bin_cnt = np.zeros(n_bins, np.int64)
    bin_members = [[] for _ in range(n_bins)]
    for b in occ:
        cand = np.where(bin_cnt < BPG)[0]
        i = cand[np.argmin(bin_rows[cand])]
        bin_members[i].append(b)
        bin_rows[i] += counts[b]
        bin_cnt[i] += 1
    assert bin_rows.max() <= 128 - APAD, "group row overflow"
    # sort bins by rows desc; slot g = bins[8g:8g+8]; snake over cores
    srt = np.argsort(-bin_rows, kind='stable')
    bins, NT = [], []
    for g in range(NGRP):
        sel = [srt[g * N_CORES + c] for c in range(N_CORES)]
        sl = [bin_members[i] for i in sel]
        if g % 2:
            sl = sl[::-1]
        bins.append(sl)
        mx = max(2, max(bin_rows[i] for i in sel))
        NT.append(int(min(128 - APAD, (mx + 1) // 2 * 2)))
    return bins, NT, tok_of_bucket


def _geom(NT):
    """Per-group kv column geometry: KCH chunks of [K^T_k | a^T_k]
    (NSP cols each), oh8h block (NT[g]+APAD cols), then 2 V halves.
    qTa chunks are [a^T_k | pad | qT_k] (NT[g]+APAD cols each); token
    rows sit at partition APAD.. everywhere before the probs transpose."""
    wgs = [KCH * NSP + NT[g] + APAD + 2 * D for g in range(NGRP)]
    kcols = np.concatenate([[0], np.cumsum(wgs)]).astype(int)
    rows = np.concatenate([[0], np.cumsum(NT)]).astype(int)
    rowsP = np.concatenate([[0], np.cumsum([n + APAD for n in NT])]).astype(int)
    qtc = np.concatenate(
        [[0], np.cumsum([KCH * (n + APAD) for n in NT])]).astype(int)
    return wgs, kcols, rows, rowsP, qtc


def _consts():
    iota8 = np.broadcast_to(np.arange(BPG, dtype=np.float32),
                            (128, BPG)).copy()
    win = (np.arange(NS)[None, :] // SPB) == np.arange(BPG)[:, None]
    winmask8 = np.where(win, 0.0, NEG).astype(np.float32)      # [BPG, NS]
    identw = np.eye(128, dtype=np.float16)
    return iota8, winmask8, identw


def _pack_core(core_bins, NT, tok_of_bucket, q_flat, tids_flat,
               KT, V, slot_tids, CBT):
    """Build this core's input arrays. core_bins[g] = list of bucket ids."""
    wgs, kcols, rows, rowsP, qtc = _geom(NT)
    i16 = int(rows[-1])
    qrow = np.zeros((int(rowsP[-1]), D), np.float16)
    qT = np.zeros((128, int(qtc[-1])), np.float16)
    side = np.full((128, 2 * NGRP), -1.0, np.float32)
    tidb = np.full((1, NGRP * NS), -2.0, np.float32)
    kv = np.zeros((128, int(kcols[-1])), np.float16)
    tok_idx = np.full(i16, -1, np.int64)

    for g in range(NGRP):
        nt, col = NT[g], int(kcols[g])
        ntp = nt + APAD
        slot_ids = np.zeros(NS, np.int64)
        real_slots = np.zeros(NS, bool)
        anchors = np.zeros((D, BPG), np.float32)
        oh8h = np.zeros((BPG, ntp), np.float32)
        qTv = qT[:, int(qtc[g]):int(qtc[g]) + KCH * ntp].reshape(128, KCH, ntp)
        r = 0
        for j, b in enumerate(core_bins[g]):
            toks = tok_of_bucket[b]
            c = len(toks)
            slot_ids[j * SPB:(j + 1) * SPB] = np.arange(b * SPB, (b + 1) * SPB)
            real_slots[j * SPB:(j + 1) * SPB] = True
            anchors[:, j] = CBT[:, b]
            tidb[0, g * NS + j * SPB:g * NS + (j + 1) * SPB] = \
                slot_tids[b * SPB:(b + 1) * SPB] + j * WENC
            qg = q_flat[toks]
            qrow[int(rowsP[g]) + APAD + r:int(rowsP[g]) + APAD + r + c] = qg
            qTv[:, :, APAD + r:APAD + r + c] = \
                qg.reshape(c, KCH, 128).transpose(2, 1, 0)
            side[APAD + r:APAD + r + c, 2 * g] = tids_flat[toks] + j * WENC
            side[APAD + r:APAD + r + c, 2 * g + 1] = j
            oh8h[j, APAD + r:APAD + r + c] = 0.5
            tok_idx[int(rows[g]) + r:int(rows[g]) + r + c] = toks
            r += c
        # anchors as leading stationary columns (a0t rows 0..7 in u_ps)
        qTv[:, :, 0:BPG] = anchors.reshape(KCH, 128, BPG).transpose(1, 0, 2)
        # ka chunks [KCH, 128, NSP]: per chunk K^T slots then anchors
        ktg = KT[:, slot_ids].reshape(KCH, 128, NS) * real_slots[None, None, :]
        atp = anchors.reshape(KCH, 128, BPG)
        ka = np.concatenate([ktg, atp], axis=2)
        c0 = col
        kv[:, c0:c0 + KCH * NSP] = \
            ka.transpose(1, 0, 2).reshape(128, -1).astype(np.float16)
        c0 += KCH * NSP
        kv[0:BPG, c0:c0 + ntp] = oh8h.astype(np.float16)
        c0 += ntp
        vb = V[slot_ids] * real_slots[:, None]          # [NS, D]
        kv[:, c0:c0 + D] = vb[0:128].astype(np.float16)
        kv[:, c0 + D:c0 + 2 * D] = vb[128:256].astype(np.float16)
    return dict(qrow=qrow, qT=qT, side=side, tidb=tidb, kv=kv), tok_idx


# ------------------------------------------------------------- device kernel

def _build_nc(NT):
    from concourse import bacc, mybir, tile

    F32 = mybir.dt.float32
    F16 = mybir.dt.float16
    AL = mybir.AluOpType
    AF = mybir.ActivationFunctionType
    X = mybir.AxisListType.X

    wgs, kcols, rows, rowsP, qtc = _geom(NT)
    i16 = int(rows[-1])

    nc = bacc.Bacc(trn_type="TRN2", target_bir_lowering=False, debug=False)
    d_kv = nc.dram_tensor("kv", [128, int(kcols[-1])], F16,
                          kind="ExternalInput").ap()
    d_qT = nc.dram_tensor("qT", [128, int(qtc[-1])], F16,
                          kind="ExternalInput").ap()
    d_qrow = nc.dram_tensor("qrow", [int(rowsP[-1]), D], F16,
                            kind="ExternalInput").ap()
    d_side = nc.dram_tensor("side", [128, 2 * NGRP], F32,
                            kind="ExternalInput").ap()
    d_tidb = nc.dram_tensor("tidb", [1, NGRP * NS], F32,
                            kind="ExternalInput").ap()
    d_iota8 = nc.dram_tensor("iota8", [128, BPG], F32,
                             kind="ExternalInput").ap()
    d_winmask8 = nc.dram_tensor("winmask8", [BPG, NS], F32,
                                kind="ExternalInput").ap()
    d_identw = nc.dram_tensor("identw", [128, 128], F16,
                              kind="ExternalInput").ap()
    d_out = nc.dram_tensor("outp", [i16, D], F16, kind="ExternalOutput").ap()
    w2max = max(wgs[2 * p] + wgs[2 * p + 1] for p in range(NGRP // 2))
    qt2max = max(int(qtc[2 * p + 2] - qtc[2 * p]) for p in range(NGRP // 2))

    with tile.TileContext(nc) as tc:
        with tc.tile_pool(name="const", bufs=1) as pc, \
             tc.tile_pool(name="kvp", bufs=4) as pkv, \
             tc.tile_pool(name="io", bufs=4) as pio, \
             tc.tile_pool(name="wk", bufs=2) as pw, \
             tc.tile_pool(name="outp", bufs=3) as po, \
             tc.tile_pool(name="psA", bufs=2, space="PSUM") as ppa, \
             tc.tile_pool(name="psB", bufs=1, space="PSUM") as ppb:

            iota8 = pc.tile([128, BPG], F32)
            winmask8 = pc.tile([BPG, NS], F32)
            identw = pc.tile([128, 128], F16)
            side_t = pc.tile([128, 2 * NGRP], F32)
            tidb_a = pc.tile([1, NGRP * NS], F32)
            nc.sync.dma_start(iota8[:], d_iota8)
            nc.sync.dma_start(winmask8[:], d_winmask8)
            nc.sync.dma_start(identw[:], d_identw)
            nc.sync.dma_start(side_t[:], d_side)
            nc.sync.dma_start(tidb_a[:], d_tidb)
            # preload the ln+exp act-function set (id 6) once; the
            # fixpoint pass then inserts no per-activation table loads
            ld = mybir.InstLoadActFuncSet(
                name=nc.get_next_instruction_name(), ins=[], outs=[],
                act_func_set_id=6)
            nc.scalar.add_instruction(ld)
            eps6 = pc.tile([128, 1], F32)
            nc.gpsimd.memset(eps6[:], 1e-6)
            tau28 = pc.tile([128, 1], F32)
            nc.gpsimd.memset(tau28[:], TAU * TAU / 8.0)
            ones256 = pc.tile([128, NS], F32)
            nc.gpsimd.memset(ones256[:], 1.0)

            kv2_t = qt2_t = None
            for g in range(NGRP):
                nt = NT[g]
                ntp = nt + APAD
                wg = wgs[g]
                r0 = int(rows[g])
                r0p = int(rowsP[g])
                voff = KCH * NSP + ntp  # V offset within this group's kv
                B0, B1 = APAD, APAD + nt  # token rows (anchors/pad at 0:32)

                if g % 2 == 0:
                    wpair = wgs[g] + wgs[g + 1]
                    qpair = int(qtc[g + 2] - qtc[g])
                    kv2_t = pkv.tile([128, w2max], F16, tag="kv")
                    nc.sync.dma_start(kv2_t[:, 0:wpair],
                                      d_kv[:, int(kcols[g]):
                                           int(kcols[g]) + wpair])
                    qt2_t = pio.tile([128, qt2max], F16, tag="qt")
                    nc.scalar.dma_start(qt2_t[:, 0:qpair],
                                        d_qT[:, int(qtc[g]):
                                             int(qtc[g]) + qpair])
                    kvo = qto = 0
                else:
                    kvo = wgs[g - 1]
                    qto = int(qtc[g] - qtc[g - 1])
                kv_t = kv2_t[:, kvo:kvo + wg]
                ka = kv_t[:, 0:KCH * NSP].rearrange("p (k s) -> p k s", k=KCH)
                qta = qt2_t[:, qto:qto + KCH * ntp].rearrange(
                    "p (k t) -> p k t", k=KCH)
                qr_t = pio.tile([128, D], F16, tag="qr")
                nc.scalar.dma_start(qr_t[0:B1, :], d_qrow[r0p:r0p + B1, :])
                tidb_t = pw.tile([128, NS], F32, tag="tidb")
                nc.gpsimd.partition_broadcast(
                    tidb_t[0:B1, :], tidb_a[0:1, g * NS:(g + 1) * NS])

                # --- mask8: row's own instance column (for q.a extraction)
                mask8 = pw.tile([128, BPG], F32, tag="mask8")
                nc.vector.tensor_scalar(out=mask8[0:B1, :],
                                        in0=iota8[0:B1, :],
                                        scalar1=side_t[0:B1,
                                                       2 * g + 1:2 * g + 2],
                                        scalar2=None, op0=AL.is_equal)

                # --- ssq = |q|^2 ; s1 = 0.5/|q| = rsqrt(4 ssq)  (GpSimd)
                scr = pw.tile([128, D], F16, tag="scr")
                ssq = pw.tile([128, 1], F32, tag="ssq")
                nc.vector.scalar_tensor_tensor(
                    out=scr[0:B1, :], in0=qr_t[0:B1, :], scalar=1.0,
                    in1=qr_t[0:B1, :], op0=AL.mult, op1=AL.mult,
                    accum_out=ssq[0:B1, :])
                s1n = pw.tile([128, 1], F32, tag="s1n")
                nc.scalar.activation(s1n[0:B1, :], ssq[0:B1, :], AF.Ln,
                                     bias=eps6[0:B1, :], scale=4.0)
                s1 = pw.tile([128, 1], F32, tag="s1")
                nc.scalar.activation(s1[0:B1, :], s1n[0:B1, :], AF.Exp,
                                     scale=-0.5)

                # --- u rows: [a0t(8) | tokens(nt)] x [K | a]
                u_ps = ppa.tile([128, NSP], F32, tag="u")
                for k in range(KCH):
                    nc.tensor.matmul(u_ps[0:ntp, :], qta[:, k, 0:ntp],
                                     ka[:, k, :], start=(k == 0),
                                     stop=(k == KCH - 1))
                a0t = pw.tile([BPG, NS], F16, tag="a0tsb")
                nc.vector.scalar_tensor_tensor(
                    out=a0t[:], in0=u_ps[0:BPG, 0:NS], scalar=1.0,
                    in1=winmask8[:], op0=AL.mult, op1=AL.add)
                c_ps = ppa.tile([128, NS], F32, tag="c")
                nc.tensor.matmul(c_ps[0:B1, :],
                                 kv_t[0:BPG, KCH * NSP:KCH * NSP + ntp],
                                 a0t[:], start=True, stop=True)

                # --- rw10 = (1/W)/tau = recip(sqrt(4*(tau^2/4)*W^2))
                qasc = pw.tile([128, BPG], F32, tag="qasc")
                qa1 = pw.tile([128, 1], F32, tag="qa1")
                nc.vector.scalar_tensor_tensor(
                    out=qasc[0:B1, :], in0=u_ps[0:B1, NS:NSP],
                    scalar=TAU * TAU / 4.0,
                    in1=mask8[0:B1, :], op0=AL.mult, op1=AL.mult,
                    accum_out=qa1[0:B1, :])
                w2 = pw.tile([128, 1], F32, tag="w2")
                nc.vector.scalar_tensor_tensor(
                    out=w2[0:B1, :], in0=qa1[0:B1, :], scalar=s1[0:B1, :],
                    in1=tau28[0:B1, :], op0=AL.mult, op1=AL.add)
                rwn = pw.tile([128, 1], F32, tag="rwn")
                nc.scalar.activation(rwn[0:B1, :], w2[0:B1, :], AF.Ln,
                                     bias=eps6[0:B1, :], scale=4.0)
                rw10 = pw.tile([128, 1], F32, tag="rw10")
                nc.scalar.activation(rw10[0:B1, :], rwn[0:B1, :], AF.Exp,
                                     scale=-0.5)

                # --- sc = u*s1 + c   (masked cols ~ -15000)
                sc1 = pw.tile([128, NS], F32, tag="sc1")
                nc.vector.tensor_scalar(out=sc1[0:B1, :],
                                        in0=u_ps[0:B1, 0:NS],
                                        scalar1=s1[0:B1, :], scalar2=None,
                                        op0=AL.mult)
                sc = pw.tile([128, NS], F32, tag="sc")
                nc.vector.tensor_tensor(out=sc[0:B1, :], in0=sc1[0:B1, :],
                                        in1=c_ps[0:B1, :], op=AL.add)
                negmax = pw.tile([128, 1], F32, tag="negmax")
                nc.vector.reduce_max(negmax[0:B1, :], sc[0:B1, :], axis=X,
                                     negate=True)
                ebias = pw.tile([128, 1], F32, tag="ebias")
                nc.vector.tensor_tensor(out=ebias[0:B1, :],
                                        in0=negmax[0:B1, :],
                                        in1=rw10[0:B1, :], op=AL.mult)
                ex = pw.tile([128, NS], F32, tag="ex")
                esum = pw.tile([128, 1], F32, tag="esum")
                nc.scalar.activation(ex[0:B1, :], sc[0:B1, :], AF.Exp,
                                     bias=ebias[0:B1, :],
                                     scale=rw10[0:B1, :],
                                     accum_out=esum[0:B1, :])
                rsum = pw.tile([128, 1], F32, tag="rsum")
                nc.vector.reciprocal(rsum[0:B1, :], esum[0:B1, :])

                # --- hard match path (window folded into tid encoding)
                match = pw.tile([128, NS], F32, tag="match")
                msum = pw.tile([128, 1], F32, tag="msum")
                nc.vector.scalar_tensor_tensor(
                    out=match[0:B1, :], in0=tidb_t[0:B1, :],
                    scalar=side_t[0:B1, 2 * g:2 * g + 1],
                    in1=ones256[0:B1, :],
                    op0=AL.is_equal, op1=AL.mult,
                    accum_out=msum[0:B1, :])
                mden = pw.tile([128, 1], F32, tag="mden")
                nc.vector.tensor_scalar(out=mden[0:B1, :], in0=msum[0:B1, :],
                                        scalar1=1e-9, scalar2=None,
                                        op0=AL.add)
                mrec = pw.tile([128, 1], F32, tag="mrec")
                nc.vector.reciprocal(mrec[0:B1, :], mden[0:B1, :])
                nohas = pw.tile([128, 1], F32, tag="nohas")
                nc.vector.tensor_scalar(out=nohas[0:B1, :],
                                        in0=msum[0:B1, :],
                                        scalar1=0.0, scalar2=None,
                                        op0=AL.is_le)
                rs_nh = pw.tile([128, 1], F32, tag="rs_nh")
                nc.vector.tensor_tensor(out=rs_nh[0:B1, :],
                                        in0=rsum[0:B1, :],
                                        in1=nohas[0:B1, :], op=AL.mult)
                hard = pw.tile([128, NS], F32, tag="hard")
                nc.vector.tensor_scalar(out=hard[0:B1, :],
                                        in0=match[0:B1, :],
                                        scalar1=mrec[0:B1, :], scalar2=None,
                                        op0=AL.mult)
                probs = pw.tile([128, NS], F16, tag="probs")
                nc.vector.scalar_tensor_tensor(
                    out=probs[0:B1, :], in0=ex[0:B1, :],
                    scalar=rs_nh[0:B1, :], in1=hard[0:B1, :],
                    op0=AL.mult, op1=AL.add)

                # --- probs^T (junk rows 0:32 become junk cols), val=probs@V
                pt_ps = ppb.tile([128, 2, 128], F16, tag="pt")
                for h in range(2):
                    nc.tensor.transpose(pt_ps[:, h, 0:B1],
                                        probs[0:B1, h * 128:(h + 1) * 128],
                                        identw[0:B1, 0:B1])
                pt = pw.tile([128, 2, 128], F16, tag="ptsb")
                for h in range(2):
                    nc.vector.tensor_copy(pt[:, h, B0:B1],
                                          pt_ps[:, h, B0:B1])
                pv = ppb.tile([128, D], F32, tag="pv")
                for j in range(2):
                    for h in range(2):
                        nc.tensor.matmul(
                            pv[0:nt, j * 512:(j + 1) * 512],
                            pt[:, h, B0:B1],
                            kv_t[:, voff + h * D + j * 512:
                                 voff + h * D + (j + 1) * 512],
                            start=(h == 0), stop=(h == 1))
                out_sb = po.tile([128, D], F16, tag="out_sb")
                nc.vector.tensor_copy(out_sb[0:nt, :], pv[0:nt, :])
                nc.sync.dma_start(d_out[r0:r0 + nt, :], out_sb[0:nt, :])
    nc.compile()
    return nc


# ------------------------------------------------------------------ emulator

def _emulate_core(ins, NT):
    """Numpy emulation of the device kernel (fp16 data, f32 math)."""
    qrow = ins["qrow"].astype(np.float32)
    qT = ins["qT"].astype(np.float32)
    side, tidb = ins["side"], ins["tidb"]
    kv = ins["kv"].astype(np.float32)
    wgs, kcols, rows, rowsP, qtc = _geom(NT)
    i16 = int(rows[-1])
    out = np.zeros((i16, D), np.float32)
    iota8, winmask8, _ = _consts()
    for g in range(NGRP):
        nt, col = NT[g], int(kcols[g])
        ntp = nt + APAD
        voff = col + KCH * NSP + ntp
        ka = kv[:, col:col + KCH * NSP].reshape(128, KCH, NSP)
        KT = ka[:, :, 0:NS].transpose(1, 0, 2).reshape(D, NS)
        AT = ka[:, :, NS:NSP].transpose(1, 0, 2).reshape(D, BPG)
        oh8h = kv[0:BPG, col + KCH * NSP:col + KCH * NSP + ntp][:, APAD:]
        vb = np.concatenate([kv[:, voff:voff + D].T,
                             kv[:, voff + D:voff + 2 * D].T], axis=1).T
        q = qrow[int(rowsP[g]) + APAD:int(rowsP[g]) + ntp]
        qTg = qT[:, int(qtc[g]):int(qtc[g]) + KCH * ntp].reshape(128, KCH,
                                                                 ntp)
        qTg = qTg.transpose(1, 0, 2).reshape(D, ntp)[:, APAD:]
        sd = side[APAD:APAD + nt, 2 * g:2 * g + 2]

        mask8 = (iota8[0:nt, :] == sd[:, 1:2]).astype(np.float32)
        ssq = (q * q).sum(-1, keepdims=True)
        s1 = 1.0 / np.sqrt(4.0 * ssq + 1e-6)
        a0t = (AT.T @ KT + winmask8).astype(np.float16).astype(np.float32)
        u = qTg.T @ np.concatenate([KT, AT], axis=1)    # [nt, NSP]
        c = oh8h.T @ a0t
        qa1 = (u[:, NS:NSP] * (TAU * TAU / 4.0) * mask8).sum(-1,
                                                             keepdims=True)
        w2 = qa1 * s1 + TAU * TAU / 8.0
        rw10 = 1.0 / np.sqrt(4.0 * w2 + 1e-6)
        sc = u[:, 0:NS] * s1 + c
        m = sc.max(-1, keepdims=True)
        ex = np.exp((sc - m) * rw10)
        esum = ex.sum(-1, keepdims=True)
        match = (tidb[0, g * NS:(g + 1) * NS][None, :] ==
                 sd[:, 0:1]).astype(np.float32)
        msum = match.sum(-1, keepdims=True)
        nohas = (msum <= 0).astype(np.float32)
        hard = match / (msum + 1e-9)
        probs = (ex * (nohas / esum) + hard).astype(np.float16)
        out[rows[g]:rows[g] + nt] = \
            (probs.astype(np.float32) @ vb).astype(np.float16)
    return out


# -------------------------------------------------------------------- kernel

def kernel(query_emb, tids, slot_keys, slot_values, slot_tids,
           centroid_codebook, _emulate=False, _trace=False):
    B, T, _ = query_emb.shape
    BT = B * T
    q_flat = np.ascontiguousarray(query_emb.reshape(BT, D), np.float32)
    tids_flat = np.asarray(tids).reshape(BT)
    st = np.asarray(slot_tids).astype(np.float32)
    KT = np.ascontiguousarray(np.asarray(slot_keys, np.float32).T)     # [D, S]
    V = np.asarray(slot_values, np.float32)
    CBT = np.ascontiguousarray(np.asarray(centroid_codebook, np.float32).T)

    bins, NT, tok_of_bucket = _routing(tids_flat)
    iota8, winmask8, identw = _consts()

    in_maps, tok_idxs = [], []
    for c in range(N_CORES):
        core_bins = [bins[g][c] for g in range(NGRP)]
        ins, tok_idx = _pack_core(core_bins, NT, tok_of_bucket, q_flat,
                                  tids_flat, KT, V, st, CBT)
        ins.update(iota8=iota8, winmask8=winmask8, identw=identw)
        in_maps.append(ins)
        tok_idxs.append(tok_idx)

    out_flat = np.zeros((BT, D), np.float32)
    if _emulate:
        for c in range(N_CORES):
            o = _emulate_core(in_maps[c], NT)
            valid = tok_idxs[c] >= 0
            out_flat[tok_idxs[c][valid]] = o[valid]
        return out_flat.reshape(B, T, D).astype(np.float32)

    _install_ntff_hook()
    from concourse import bass_utils
    key = tuple(NT)
    if key not in _COMPILED:
        _COMPILED[key] = _build_nc(NT)
    nc = _COMPILED[key]
    res = bass_utils.run_bass_kernel_spmd(
        nc, in_maps, core_ids=list(range(N_CORES)), trace=_trace)
    for c in range(N_CORES):
        o = np.asarray(res.results[c]["outp"], np.float32)
        valid = tok_idxs[c] >= 0
        out_flat[tok_idxs[c][valid]] = o[valid]
    out = out_flat.reshape(B, T, D).astype(np.float32)
    if _trace:
        kernel._last_exec_time_ns = res.exec_time_ns
        kernel._last_results = res
    return out
